# revision 46
# baseline (speedup 1.0000x reference)
"""Trainium2 Bass kernel for nn_DisentangledSelfAttention.

Sharding: batch (B=8) across the 8 NeuronCores, weights replicated.

Host-side algebra (exact identities, done in numpy inside kernel()):
  q = relu(x @ (W_Q @ Wq_w.T) + bq)   -- no nonlinearity between the two
  k = relu(x @ (W_K @ Wk_w.T) + bk)      projection stages, so they fold
  v = relu(x @ (W_V @ Wv_w.T) + bv)      into one [E, A] matrix each
  uw = softmax_l(x @ (W_K @ Wu_w.T) + bu)   (tiny: E*H mults)
x is shipped pre-transposed (xT [E, L]) and pre-rounded to bf16; the
folded weights are shipped bf16 pre-chunked for direct use as lhsT.

Device math per core (one batch item; L=1024, A=512, H=8, HD=64):
  qT/kT = relu(W.T @ xT + b)  [A, L]   (lhsT = W_eff natural, rhs = xT)
  v     = relu(x @ Wv + b)    [L, A]   (lhsT = xT, rhs = Wv_eff)
  The torch .view group reshape makes attention block-diagonal over
  128-row l-blocks (group g), with pseudo-seq s -> (l=128g+r, a=64c+d).
  We enumerate s as (par, ac, r) with c = 2*ac+par so that qT partition
  halves are directly the pair-matmul rhs.  kdup duplicates kT group
  slices into both partition halves (slot t = head-col c'=t) so the
  pair lhsT is available at either base partition; free-dim reduction
  on kdup gives the true group mean (both halves hold all of c').
  k is mean-centered in place; q centering folds into the exp bias:
     S_centered = q . kc - (sum_s q/1024) . kc[s']   (per-s'-row bias)
  PT[t] = exp(S/8 + ebias) (bf16), then out[s-block, d] accumulates
  PT[t][:, block].T @ v[:, 64t:64t+64] (N=64 moving dim), denominators
  via rhs=ones K=128 matmuls, unary rank-1 term via host uw:
     out = psum * (1/den) + uwv   in one DVE scalar_tensor_tensor.
"""

import os
import numpy as np

B, L, E, A, H, HD = 8, 1024, 1024, 512, 8, 64
G = 8
N_CORES = 8

SHIFT_DMA = os.environ.get("KERNEL_SHIFT_DMA", "0") == "1"


def _build_nc():
    from contextlib import ExitStack

    import concourse.bass as bass
    import concourse.tile as tile
    import concourse.mybir as mybir
    from concourse import bacc

    f32 = mybir.dt.float32
    bf16 = mybir.dt.bfloat16
    X = mybir.AxisListType.X
    XY = mybir.AxisListType.XY
    Alu = mybir.AluOpType
    Act = mybir.ActivationFunctionType

    nc = bacc.Bacc("TRN2", target_bir_lowering=False, debug=False,
                   num_devices=N_CORES)

    xT_d = nc.dram_tensor("xT", [E, L], bf16, kind="ExternalInput").ap()
    Wq_d = nc.dram_tensor("Wq", [128, 8, A], bf16, kind="ExternalInput").ap()
    Wk_d = nc.dram_tensor("Wk", [128, 8, A], bf16, kind="ExternalInput").ap()
    Wv_d = nc.dram_tensor("Wv", [128, 8, A], bf16, kind="ExternalInput").ap()
    bq_d = nc.dram_tensor("bq", [128, 4], f32, kind="ExternalInput").ap()
    bk_d = nc.dram_tensor("bk", [128, 4], f32, kind="ExternalInput").ap()
    bv_d = nc.dram_tensor("bv", [1, A], bf16, kind="ExternalInput").ap()
    uc_d = nc.dram_tensor("ucol", [128, G, H], bf16, kind="ExternalInput").ap()
    out_d = nc.dram_tensor("out", [L, A], f32, kind="ExternalOutput").ap()

    with tile.TileContext(nc) as tc, ExitStack() as ctx:
        persist = ctx.enter_context(tc.tile_pool(name="persist", bufs=1))
        pt_pool = ctx.enter_context(tc.tile_pool(name="pt", bufs=2))
        ost_pool = ctx.enter_context(tc.tile_pool(name="ost", bufs=2))
        small = ctx.enter_context(tc.tile_pool(name="small", bufs=24))
        p_pair = ctx.enter_context(tc.tile_pool(name="p_pair", bufs=2, space="PSUM"))
        p_o = ctx.enter_context(tc.tile_pool(name="p_o", bufs=2, space="PSUM"))
        p_sm = ctx.enter_context(tc.tile_pool(name="p_sm", bufs=2, space="PSUM"))

        ones_row = persist.tile([1, 128], bf16, tag="ones_row")
        nc.vector.memset(ones_row, 1.0)
        ones_col = persist.tile([128, 1], bf16, tag="ones_col")
        nc.vector.memset(ones_col, 1.0)
        zeros = persist.tile([128, 256], bf16, tag="zeros")
        nc.vector.memset(zeros, 0.0)

        xT = persist.tile([128, 8, L], bf16, tag="xT")
        wk_sb = persist.tile([128, 8, A], bf16, tag="wk_sb")
        wq_sb = persist.tile([128, 8, A], bf16, tag="wq_sb")
        wv_sb = persist.tile([128, 8, A], bf16, tag="wv_sb")
        x_src = xT_d.rearrange("(ec p) l -> p ec l", p=128)
        nc.sync.dma_start(wk_sb[:, 0:1, :], Wk_d[:, 0:1, :])
        nc.sync.dma_start(xT[:, 0:1, 0:256], x_src[:, 0:1, 0:256])
        nc.sync.dma_start(wk_sb[:, 1:4, :], Wk_d[:, 1:4, :])
        nc.sync.dma_start(xT[:, 1:4, 0:256], x_src[:, 1:4, 0:256])
        nc.sync.dma_start(wk_sb[:, 4:8, :], Wk_d[:, 4:8, :])
        nc.sync.dma_start(xT[:, 4:8, 0:256], x_src[:, 4:8, 0:256])
        bk_sb = persist.tile([128, 4], f32, tag="bk_sb")
        nc.gpsimd.dma_start(bk_sb, bk_d)
        nc.sync.dma_start(wq_sb[:, 0:4, :], Wq_d[:, 0:4, :])
        bq_sb = persist.tile([128, 4], f32, tag="bq_sb")
        nc.gpsimd.dma_start(bq_sb, bq_d)
        nc.sync.dma_start(wq_sb[:, 4:8, :], Wq_d[:, 4:8, :])
        nc.sync.dma_start(wv_sb[:, 0:4, :], Wv_d[:, 0:4, :])
        nc.sync.dma_start(wv_sb[:, 4:8, :], Wv_d[:, 4:8, :])
        bv_sb = persist.tile([1, A], bf16, tag="bv_sb")
        nc.gpsimd.dma_start(bv_sb, bv_d)
        uc_sb = persist.tile([128, G, H], bf16, tag="uc_sb")
        nc.gpsimd.dma_start(uc_sb, uc_d)
        nc.sync.dma_start(xT[:, :, 256:512], x_src[:, :, 256:512])
        # (lq2/lq3 windows are prefetched inside proj_block)

        qT = persist.tile([128, 4, L], bf16, tag="qT")
        kT = persist.tile([128, 4, L], bf16, tag="kT")
        kdup = persist.tile([128, G, 8, 128], bf16, tag="kdup")
        v_all = persist.tile([128, 8, A], bf16, tag="v_all")
        qpartb = persist.tile([128, G], bf16, tag="qpartb")
        ebias = persist.tile([128, G, 8], f32, tag="ebias")

        # kdup views: [p, two, tt, g, r] with t-slot = 2*tt + two
        kdup_v = kdup.rearrange("p g (tt two) r -> p two tt g r", two=2)

        def qk_chain(w_sb, b_sb, dst, lq, ac, on_act=False, accs=None):
            # dst[:, ac, 256lq:+256] = relu(W.T @ xT + b), N=256 matmuls
            ps = p_pair.tile([128, 1024], f32, tag="pair",
                             name=f"qk_{lq}_{ac}")
            for ec in range(8):
                nc.tensor.matmul(
                    ps[:, 0:256], w_sb[:, ec, 128 * ac:128 * ac + 128],
                    xT[:, ec, 256 * lq:256 * lq + 256],
                    start=(ec == 0), stop=(ec == 7))
            if on_act:
                nc.scalar.activation(
                    out=dst[:, ac, 256 * lq:256 * lq + 256],
                    in_=ps[:, 0:256], func=Act.Relu,
                    bias=b_sb[:, ac:ac + 1], scale=1.0)
            else:
                for j in range(2):
                    nc.vector.scalar_tensor_tensor(
                        out=dst[:, ac, 256 * lq + 128 * j:256 * lq + 128 * j + 128],
                        in0=ps[:, 128 * j:128 * j + 128],
                        scalar=b_sb[:, ac:ac + 1], in1=zeros[:, 0:128],
                        op0=Alu.add, op1=Alu.max,
                        accum_out=accs[j][:, ac:ac + 1])

        def v_proj(lt):
            # v_all[:, lt, :] = relu(x @ Wv + bv): lhsT = xT l-chunk
            ps = p_o.tile([128, 8, 64], f32, tag="ps_o",
                          name=f"vps_{lt}").rearrange("p a b -> p (a b)")
            for ec in range(8):
                nc.tensor.matmul(
                    ps, xT[:, ec, 128 * lt:128 * lt + 128],
                    wv_sb[:, ec, :], start=(ec == 0), stop=False)
            nc.tensor.matmul(ps, ones_row, bv_sb, start=False, stop=True)
            nc.vector.tensor_scalar_max(v_all[:, lt, :], ps, 0.0)

        def kdup_fill(lq):
            # duplicate kT group slices into both partition halves of kdup
            sl = slice(256 * lq, 256 * lq + 256)
            gs = slice(2 * lq, 2 * lq + 2)
            src_lo = kT[0:64, :, sl].rearrange("p ac (g r) -> p ac g r", r=128)
            src_hi = kT[64:128, :, sl].rearrange("p ac (g r) -> p ac g r", r=128)
            nc.vector.tensor_copy(out=kdup_v[0:64, 0, :, gs, :], in_=src_lo)
            nc.vector.tensor_copy(out=kdup_v[64:128, 1, :, gs, :], in_=src_hi)
            if SHIFT_DMA:
                nc.sync.dma_start(kdup_v[64:128, 0, :, gs, :], src_lo)
                nc.sync.dma_start(kdup_v[0:64, 1, :, gs, :], src_hi)
            else:
                nc.vector.tensor_copy(out=kdup_v[64:128, 0, :, gs, :], in_=src_lo)
                nc.vector.tensor_copy(out=kdup_v[0:64, 1, :, gs, :], in_=src_hi)

        def prep_k(g):
            # center k (in place on kdup; free dim covers the whole group)
            mean = small.tile([128, 1], f32, tag="mean", name=f"mean_{g}")
            nc.vector.reduce_sum(mean, kdup[:, g], axis=XY)
            nc.vector.tensor_scalar_mul(mean, mean, 1.0 / 1024.0)
            nc.vector.tensor_scalar_sub(kdup[:, g], kdup[:, g], mean)

        def prep_group(g, ps_sm, qaccs):
            # q group sums (both halves), then ebias[s'] = -(sum_q . kc)/8192
            qa = qaccs[g % 2]
            t1 = small.tile([128, 1], f32, tag="t1", name=f"t1_{g}")
            t2 = small.tile([128, 1], f32, tag="t2", name=f"t2_{g}")
            nc.gpsimd.tensor_add(t1, qa[:, 0:1], qa[:, 1:2])
            nc.gpsimd.tensor_add(t2, qa[:, 2:3], qa[:, 3:4])
            nc.gpsimd.tensor_add(qpartb[:, g:g + 1], t1, t2)
            ps_b = ps_sm[:, 0:8]
            for t in range(8):
                nc.tensor.matmul(ps_b[:, t:t + 1], kdup[:, g, t, :],
                                 qpartb[:, g:g + 1], start=(t == 0), stop=(t == 7))
            nc.vector.tensor_scalar_mul(ebias[:, g, :], ps_b, -0.125 / 1024.0)

        def main_group(g, ps_sm, fillers=()):
            fillers = list(fillers)
            PT = pt_pool.tile([128, 8, 1024], bf16, tag="PT", name=f"PT_{g}")
            ps_den = ps_sm[:, 8:16]
            ps_ot = p_o.tile([128, 8, 64], f32, tag="ps_o", name=f"o_{g}")
            ps_os = [ps_ot[:, h, :] for h in range(8)]

            def emit_uwv():
                # unary rank-1 term: uwv = sum_t uw[:, t] . v[:, 64t:+64]
                ps_uwv = ps_sm[0:1, 80:144]
                for t in range(8):
                    nc.tensor.matmul(ps_uwv, uc_sb[:, g, t:t + 1],
                                     v_all[:, g, 64 * t:64 * t + 64],
                                     start=(t == 0), stop=(t == 7))
                uwv_sb = small.tile([1, 64], bf16, tag="uwv_sb",
                                    name=f"uwvs_{g}")
                nc.vector.tensor_copy(out=uwv_sb, in_=ps_uwv)
                ps_ubc = ps_sm[:, 16:80]
                nc.tensor.matmul(ps_ubc, ones_row, uwv_sb, start=True, stop=True)
                ubc_sb = small.tile([128, 64], f32, tag="ubc_sb",
                                    name=f"ubcs_{g}")
                nc.vector.tensor_copy(out=ubc_sb, in_=ps_ubc)
                return ubc_sb

            ubc_sb = None
            for t in range(8):
                ps_S = p_pair.tile([128, 1024], f32, tag="pair",
                                   name=f"S_{g}_{t}")
                nc.tensor.matmul(ps_S[:, 0:512], kdup[0:64, g, t, :],
                                 qT[0:64, :, 128 * g:128 * g + 128],
                                 start=True, stop=True)
                nc.tensor.matmul(ps_S[:, 512:1024], kdup[64:128, g, t, :],
                                 qT[64:128, :, 128 * g:128 * g + 128],
                                 start=True, stop=True)
                nc.scalar.activation(
                    out=PT[:, t, :], in_=ps_S, func=Act.Exp,
                    bias=ebias[:, g, t:t + 1], scale=0.125)
                if t == 0:
                    ubc_sb = emit_uwv()
                elif t >= 2 and fillers:
                    fillers.pop(0)()
                for h in range(8):
                    nc.tensor.matmul(
                        ps_os[h], PT[:, t, 128 * h:128 * h + 128],
                        v_all[:, g, 64 * t:64 * t + 64],
                        start=(t == 0 and h == 0), stop=(t == 7 and h == 7))
                for h in range(8):
                    nc.tensor.matmul(
                        ps_den[:, h:h + 1], PT[:, t, 128 * h:128 * h + 128],
                        ones_col, start=(t == 0 and h == 0),
                        stop=(t == 7 and h == 7))

            while fillers:
                fillers.pop(0)()

            def tail():
                rcol = small.tile([128, 8], f32, tag="rcol", name=f"rcol_{g}")
                nc.vector.reciprocal(out=rcol, in_=ps_den)
                ostage = ost_pool.tile([128, A], f32, tag="ostage",
                                       name=f"ost_{g}")
                for c in range(8):
                    h = (c % 2) * 4 + c // 2
                    nc.vector.scalar_tensor_tensor(
                        out=ostage[:, 64 * c:64 * c + 64], in0=ps_os[h],
                        scalar=rcol[:, h:h + 1], in1=ubc_sb,
                        op0=Alu.mult, op1=Alu.add)
                    if c == 3:
                        nc.sync.dma_start(out_d[128 * g:128 * g + 128, 0:256],
                                          ostage[:, 0:256])
                nc.sync.dma_start(out_d[128 * g:128 * g + 128, 256:512],
                                  ostage[:, 256:512])
            return tail

        qaccs_by_lq = {}

        def mk_qaccs(lq):
            qaccs_by_lq[lq] = [
                small.tile([128, 4], f32, tag="qacc", name=f"qacc_{2*lq+j}")
                for j in range(2)]

        # prologue: lq0 projections emitted directly
        for ac in range(4):
            qk_chain(wk_sb, bk_sb, kT, 0, ac, on_act=True)
        mk_qaccs(0)
        for ac in range(4):
            qk_chain(wq_sb, bq_sb, qT, 0, ac, accs=qaccs_by_lq[0])
        kdup_fill(0)
        prep_k(0)
        prep_k(1)
        v_proj(0)
        v_proj(1)

        sms = {}

        def mk_preps(lq):
            sms[2 * lq] = p_sm.tile([128, 144], f32, tag="ps_sm",
                                    name=f"sm_{2*lq}")
            prep_group(2 * lq, sms[2 * lq], qaccs_by_lq[lq])
            sms[2 * lq + 1] = p_sm.tile([128, 144], f32, tag="ps_sm",
                                        name=f"sm_{2*lq+1}")
            prep_group(2 * lq + 1, sms[2 * lq + 1], qaccs_by_lq[lq])

        mk_preps(0)
        for lq in range(4):
            if lq < 2:
                w0 = 512 + 256 * lq
                nc.sync.dma_start(xT[:, :, w0:w0 + 256], x_src[:, :, w0:w0 + 256])
            if lq < 3:
                nlq = lq + 1
                mk_qaccs(nlq)
                kf = [(lambda ac=ac: qk_chain(wk_sb, bk_sb, kT, nlq, ac,
                                              on_act=True))
                      for ac in range(4)]
                qf = [(lambda ac=ac: qk_chain(wq_sb, bq_sb, qT, nlq, ac,
                                              accs=qaccs_by_lq[nlq]))
                      for ac in range(4)]
                fill0 = kf
                fill1 = (qf
                         + [lambda: mk_preps(nlq),
                            lambda: v_proj(2 * nlq), lambda: v_proj(2 * nlq + 1)])
            else:
                fill0, fill1 = [], []

            tail0 = main_group(2 * lq, sms[2 * lq], fill0)
            if lq < 3:
                kdup_fill(lq + 1)
                prep_k(2 * lq + 2)
                prep_k(2 * lq + 3)
            tail0()
            tail1 = main_group(2 * lq + 1, sms[2 * lq + 1], fill1)
            tail1()

    nc.compile()
    return nc


def _host_prep(inputs):
    import ml_dtypes
    bf = ml_dtypes.bfloat16
    f32 = np.float32
    g = {k: np.asarray(v, dtype=f32) for k, v in inputs.items()}
    Wq_eff = g["W_Q"] @ g["Wq_w"].T          # [E, A]
    Wk_eff = g["W_K"] @ g["Wk_w"].T
    Wv_eff = g["W_V"] @ g["Wv_w"].T
    Wu_eff = g["W_K"] @ g["Wu_w"].T          # [E, H]

    def chunk_w(w):  # [E, A] -> [128, 8, A] with [p, ec, a] = w[128*ec+p, a]
        return np.ascontiguousarray(
            w.reshape(8, 128, A).transpose(1, 0, 2)).astype(bf)

    wq, wk, wv = chunk_w(Wq_eff), chunk_w(Wk_eff), chunk_w(Wv_eff)
    bq = np.ascontiguousarray(g["Wq_b"].reshape(4, 128).T)
    bk = np.ascontiguousarray(g["Wk_b"].reshape(4, 128).T)
    bv = g["Wv_b"].reshape(1, A).astype(bf)

    x = g["x"]                                # [B, L, E]
    unary = np.einsum("ble,eh->blh", x, Wu_eff) + g["Wu_b"]
    unary -= unary.max(axis=1, keepdims=True)
    eu = np.exp(unary)
    uw = eu / eu.sum(axis=1, keepdims=True)   # [B, L, H]

    per_core = []
    for b in range(B):
        xT = np.ascontiguousarray(x[b].T).astype(bf)
        ucol = np.ascontiguousarray(
            uw[b].reshape(G, 128, H).transpose(1, 0, 2)).astype(bf)
        per_core.append(dict(xT=xT, Wq=wq, Wk=wk, Wv=wv, bq=bq, bk=bk,
                             bv=bv, ucol=ucol))
    return per_core


_NC_CACHE = {}


def kernel(**inputs):
    from concourse.bass_utils import run_bass_kernel_spmd

    if "nc" not in _NC_CACHE:
        _NC_CACHE["nc"] = _build_nc()
    nc = _NC_CACHE["nc"]

    in_maps = _host_prep(inputs)

    trace = os.environ.get("KERNEL_TRACE", "0") == "1"
    # First execution after a fresh NEFF load occasionally hits a transient
    # NRT_EXEC_UNIT_UNRECOVERABLE; a retry on the reloaded device succeeds.
    last_exc = None
    for _attempt in range(3):
        try:
            res = run_bass_kernel_spmd(nc, in_maps,
                                       core_ids=list(range(N_CORES)),
                                       trace=trace)
            break
        except Exception as e:
            last_exc = e
    else:
        raise last_exc
    if trace and res.exec_time_ns is not None:
        print(f"HW exec time: {res.exec_time_ns} ns")
        kernel.last_exec_time_ns = res.exec_time_ns
    out = np.stack([r["out"] for r in res.results], axis=0)
    return out


# revision 47
# speedup vs baseline: 1.0002x; 1.0002x over previous
"""Trainium2 Bass kernel for nn_DisentangledSelfAttention.

Sharding: batch (B=8) across the 8 NeuronCores, weights replicated.

Host-side algebra (exact identities, done in numpy inside kernel()):
  q = relu(x @ (W_Q @ Wq_w.T) + bq)   -- no nonlinearity between the two
  k = relu(x @ (W_K @ Wk_w.T) + bk)      projection stages, so they fold
  v = relu(x @ (W_V @ Wv_w.T) + bv)      into one [E, A] matrix each
  uw = softmax_l(x @ (W_K @ Wu_w.T) + bu)   (tiny: E*H mults)
x is shipped pre-transposed (xT [E, L]) and pre-rounded to bf16; the
folded weights are shipped bf16 pre-chunked for direct use as lhsT.

Device math per core (one batch item; L=1024, A=512, H=8, HD=64):
  qT/kT = relu(W.T @ xT + b)  [A, L]   (lhsT = W_eff natural, rhs = xT)
  v     = relu(x @ Wv + b)    [L, A]   (lhsT = xT, rhs = Wv_eff)
  The torch .view group reshape makes attention block-diagonal over
  128-row l-blocks (group g), with pseudo-seq s -> (l=128g+r, a=64c+d).
  We enumerate s as (par, ac, r) with c = 2*ac+par so that qT partition
  halves are directly the pair-matmul rhs.  kdup duplicates kT group
  slices into both partition halves (slot t = head-col c'=t) so the
  pair lhsT is available at either base partition; free-dim reduction
  on kdup gives the true group mean (both halves hold all of c').
  k-centering cancels in the softmax (it only shifts logits by a
  per-query constant), so k is used raw; q centering folds into the
  exp bias:  S = q . k - (sum_s q/1024) . k[s']   (per-s'-row bias)
  PT[t] = exp(S/8 + ebias) (bf16), then out[s-block, d] accumulates
  PT[t][:, block].T @ v[:, 64t:64t+64] (N=64 moving dim), denominators
  via rhs=ones K=128 matmuls, unary rank-1 term via host uw:
     out = psum * (1/den) + uwv   in one DVE scalar_tensor_tensor.
"""

import os
import numpy as np

B, L, E, A, H, HD = 8, 1024, 1024, 512, 8, 64
G = 8
N_CORES = 8

SHIFT_DMA = os.environ.get("KERNEL_SHIFT_DMA", "0") == "1"


def _build_nc():
    from contextlib import ExitStack

    import concourse.bass as bass
    import concourse.tile as tile
    import concourse.mybir as mybir
    from concourse import bacc

    f32 = mybir.dt.float32
    bf16 = mybir.dt.bfloat16
    X = mybir.AxisListType.X
    XY = mybir.AxisListType.XY
    Alu = mybir.AluOpType
    Act = mybir.ActivationFunctionType

    nc = bacc.Bacc("TRN2", target_bir_lowering=False, debug=False,
                   num_devices=N_CORES)

    xT_d = nc.dram_tensor("xT", [E, L], bf16, kind="ExternalInput").ap()
    Wq_d = nc.dram_tensor("Wq", [128, 8, A], bf16, kind="ExternalInput").ap()
    Wk_d = nc.dram_tensor("Wk", [128, 8, A], bf16, kind="ExternalInput").ap()
    Wv_d = nc.dram_tensor("Wv", [128, 8, A], bf16, kind="ExternalInput").ap()
    bq_d = nc.dram_tensor("bq", [128, 4], f32, kind="ExternalInput").ap()
    bk_d = nc.dram_tensor("bk", [128, 4], f32, kind="ExternalInput").ap()
    bv_d = nc.dram_tensor("bv", [1, A], bf16, kind="ExternalInput").ap()
    uc_d = nc.dram_tensor("ucol", [128, G, H], bf16, kind="ExternalInput").ap()
    out_d = nc.dram_tensor("out", [L, A], f32, kind="ExternalOutput").ap()

    with tile.TileContext(nc) as tc, ExitStack() as ctx:
        persist = ctx.enter_context(tc.tile_pool(name="persist", bufs=1))
        pt_pool = ctx.enter_context(tc.tile_pool(name="pt", bufs=2))
        ost_pool = ctx.enter_context(tc.tile_pool(name="ost", bufs=2))
        small = ctx.enter_context(tc.tile_pool(name="small", bufs=24))
        p_pair = ctx.enter_context(tc.tile_pool(name="p_pair", bufs=2, space="PSUM"))
        p_o = ctx.enter_context(tc.tile_pool(name="p_o", bufs=2, space="PSUM"))
        p_sm = ctx.enter_context(tc.tile_pool(name="p_sm", bufs=2, space="PSUM"))

        ones_row = persist.tile([1, 128], bf16, tag="ones_row")
        nc.vector.memset(ones_row, 1.0)
        ones_col = persist.tile([128, 1], bf16, tag="ones_col")
        nc.vector.memset(ones_col, 1.0)
        zeros = persist.tile([128, 256], bf16, tag="zeros")
        nc.vector.memset(zeros, 0.0)

        xT = persist.tile([128, 8, L], bf16, tag="xT")
        wk_sb = persist.tile([128, 8, A], bf16, tag="wk_sb")
        wq_sb = persist.tile([128, 8, A], bf16, tag="wq_sb")
        wv_sb = persist.tile([128, 8, A], bf16, tag="wv_sb")
        x_src = xT_d.rearrange("(ec p) l -> p ec l", p=128)
        nc.sync.dma_start(wk_sb[:, 0:1, :], Wk_d[:, 0:1, :])
        nc.sync.dma_start(xT[:, 0:1, 0:256], x_src[:, 0:1, 0:256])
        nc.sync.dma_start(wk_sb[:, 1:4, :], Wk_d[:, 1:4, :])
        nc.sync.dma_start(xT[:, 1:4, 0:256], x_src[:, 1:4, 0:256])
        nc.sync.dma_start(wk_sb[:, 4:8, :], Wk_d[:, 4:8, :])
        nc.sync.dma_start(xT[:, 4:8, 0:256], x_src[:, 4:8, 0:256])
        bk_sb = persist.tile([128, 4], f32, tag="bk_sb")
        nc.gpsimd.dma_start(bk_sb, bk_d)
        nc.sync.dma_start(wq_sb[:, 0:4, :], Wq_d[:, 0:4, :])
        bq_sb = persist.tile([128, 4], f32, tag="bq_sb")
        nc.gpsimd.dma_start(bq_sb, bq_d)
        nc.sync.dma_start(wq_sb[:, 4:8, :], Wq_d[:, 4:8, :])
        nc.sync.dma_start(wv_sb[:, 0:4, :], Wv_d[:, 0:4, :])
        nc.sync.dma_start(wv_sb[:, 4:8, :], Wv_d[:, 4:8, :])
        bv_sb = persist.tile([1, A], bf16, tag="bv_sb")
        nc.gpsimd.dma_start(bv_sb, bv_d)
        uc_sb = persist.tile([128, G, H], bf16, tag="uc_sb")
        nc.gpsimd.dma_start(uc_sb, uc_d)
        nc.sync.dma_start(xT[:, :, 256:512], x_src[:, :, 256:512])
        # (lq2/lq3 windows are prefetched inside proj_block)

        qT = persist.tile([128, 4, L], bf16, tag="qT")
        kT = persist.tile([128, 4, L], bf16, tag="kT")
        kdup = persist.tile([128, G, 8, 128], bf16, tag="kdup")
        v_all = persist.tile([128, 8, A], bf16, tag="v_all")
        qpartb = persist.tile([128, G], bf16, tag="qpartb")
        ebias = persist.tile([128, G, 8], f32, tag="ebias")

        # kdup views: [p, two, tt, g, r] with t-slot = 2*tt + two
        kdup_v = kdup.rearrange("p g (tt two) r -> p two tt g r", two=2)

        def qk_chain(w_sb, b_sb, dst, lq, ac, on_act=False, accs=None):
            # dst[:, ac, 256lq:+256] = relu(W.T @ xT + b), N=256 matmuls
            ps = p_pair.tile([128, 1024], f32, tag="pair",
                             name=f"qk_{lq}_{ac}")
            for ec in range(8):
                nc.tensor.matmul(
                    ps[:, 0:256], w_sb[:, ec, 128 * ac:128 * ac + 128],
                    xT[:, ec, 256 * lq:256 * lq + 256],
                    start=(ec == 0), stop=(ec == 7))
            if on_act:
                nc.scalar.activation(
                    out=dst[:, ac, 256 * lq:256 * lq + 256],
                    in_=ps[:, 0:256], func=Act.Relu,
                    bias=b_sb[:, ac:ac + 1], scale=1.0)
            else:
                for j in range(2):
                    nc.vector.scalar_tensor_tensor(
                        out=dst[:, ac, 256 * lq + 128 * j:256 * lq + 128 * j + 128],
                        in0=ps[:, 128 * j:128 * j + 128],
                        scalar=b_sb[:, ac:ac + 1], in1=zeros[:, 0:128],
                        op0=Alu.add, op1=Alu.max,
                        accum_out=accs[j][:, ac:ac + 1])

        def v_proj(lt):
            # v_all[:, lt, :] = relu(x @ Wv + bv): lhsT = xT l-chunk
            ps = p_o.tile([128, 8, 64], f32, tag="ps_o",
                          name=f"vps_{lt}").rearrange("p a b -> p (a b)")
            for ec in range(8):
                nc.tensor.matmul(
                    ps, xT[:, ec, 128 * lt:128 * lt + 128],
                    wv_sb[:, ec, :], start=(ec == 0), stop=False)
            nc.tensor.matmul(ps, ones_row, bv_sb, start=False, stop=True)
            nc.vector.tensor_scalar_max(v_all[:, lt, :], ps, 0.0)

        def kdup_fill(lq):
            # duplicate kT group slices into both partition halves of kdup
            sl = slice(256 * lq, 256 * lq + 256)
            gs = slice(2 * lq, 2 * lq + 2)
            src_lo = kT[0:64, :, sl].rearrange("p ac (g r) -> p ac g r", r=128)
            src_hi = kT[64:128, :, sl].rearrange("p ac (g r) -> p ac g r", r=128)
            nc.vector.tensor_copy(out=kdup_v[0:64, 0, :, gs, :], in_=src_lo)
            nc.vector.tensor_copy(out=kdup_v[64:128, 1, :, gs, :], in_=src_hi)
            if SHIFT_DMA:
                nc.sync.dma_start(kdup_v[64:128, 0, :, gs, :], src_lo)
                nc.sync.dma_start(kdup_v[0:64, 1, :, gs, :], src_hi)
            else:
                nc.vector.tensor_copy(out=kdup_v[64:128, 0, :, gs, :], in_=src_lo)
                nc.vector.tensor_copy(out=kdup_v[0:64, 1, :, gs, :], in_=src_hi)

        def prep_group(g, ps_sm, qaccs):
            # q group sums (both halves), then ebias[s'] = -(sum_q . kc)/8192
            qa = qaccs[g % 2]
            t1 = small.tile([128, 1], f32, tag="t1", name=f"t1_{g}")
            t2 = small.tile([128, 1], f32, tag="t2", name=f"t2_{g}")
            nc.gpsimd.tensor_add(t1, qa[:, 0:1], qa[:, 1:2])
            nc.gpsimd.tensor_add(t2, qa[:, 2:3], qa[:, 3:4])
            nc.gpsimd.tensor_add(qpartb[:, g:g + 1], t1, t2)
            ps_b = ps_sm[:, 0:8]
            for t in range(8):
                nc.tensor.matmul(ps_b[:, t:t + 1], kdup[:, g, t, :],
                                 qpartb[:, g:g + 1], start=(t == 0), stop=(t == 7))
            nc.vector.tensor_scalar_mul(ebias[:, g, :], ps_b, -0.125 / 1024.0)

        def main_group(g, ps_sm, fillers=()):
            fillers = list(fillers)
            PT = pt_pool.tile([128, 8, 1024], bf16, tag="PT", name=f"PT_{g}")
            ps_den = ps_sm[:, 8:16]
            ps_ot = p_o.tile([128, 8, 64], f32, tag="ps_o", name=f"o_{g}")
            ps_os = [ps_ot[:, h, :] for h in range(8)]

            def emit_uwv():
                # unary rank-1 term: uwv = sum_t uw[:, t] . v[:, 64t:+64]
                ps_uwv = ps_sm[0:1, 80:144]
                for t in range(8):
                    nc.tensor.matmul(ps_uwv, uc_sb[:, g, t:t + 1],
                                     v_all[:, g, 64 * t:64 * t + 64],
                                     start=(t == 0), stop=(t == 7))
                uwv_sb = small.tile([1, 64], bf16, tag="uwv_sb",
                                    name=f"uwvs_{g}")
                nc.vector.tensor_copy(out=uwv_sb, in_=ps_uwv)
                ps_ubc = ps_sm[:, 16:80]
                nc.tensor.matmul(ps_ubc, ones_row, uwv_sb, start=True, stop=True)
                ubc_sb = small.tile([128, 64], f32, tag="ubc_sb",
                                    name=f"ubcs_{g}")
                nc.vector.tensor_copy(out=ubc_sb, in_=ps_ubc)
                return ubc_sb

            ubc_sb = None
            for t in range(8):
                ps_S = p_pair.tile([128, 1024], f32, tag="pair",
                                   name=f"S_{g}_{t}")
                nc.tensor.matmul(ps_S[:, 0:512], kdup[0:64, g, t, :],
                                 qT[0:64, :, 128 * g:128 * g + 128],
                                 start=True, stop=True)
                nc.tensor.matmul(ps_S[:, 512:1024], kdup[64:128, g, t, :],
                                 qT[64:128, :, 128 * g:128 * g + 128],
                                 start=True, stop=True)
                nc.scalar.activation(
                    out=PT[:, t, :], in_=ps_S, func=Act.Exp,
                    bias=ebias[:, g, t:t + 1], scale=0.125)
                if t == 0:
                    ubc_sb = emit_uwv()
                elif t >= 2 and fillers:
                    fillers.pop(0)()
                for h in range(8):
                    nc.tensor.matmul(
                        ps_os[h], PT[:, t, 128 * h:128 * h + 128],
                        v_all[:, g, 64 * t:64 * t + 64],
                        start=(t == 0 and h == 0), stop=(t == 7 and h == 7))
                for h in range(8):
                    nc.tensor.matmul(
                        ps_den[:, h:h + 1], PT[:, t, 128 * h:128 * h + 128],
                        ones_col, start=(t == 0 and h == 0),
                        stop=(t == 7 and h == 7))

            while fillers:
                fillers.pop(0)()

            def tail():
                rcol = small.tile([128, 8], f32, tag="rcol", name=f"rcol_{g}")
                nc.vector.reciprocal(out=rcol, in_=ps_den)
                ostage = ost_pool.tile([128, A], f32, tag="ostage",
                                       name=f"ost_{g}")
                for c in range(8):
                    h = (c % 2) * 4 + c // 2
                    nc.vector.scalar_tensor_tensor(
                        out=ostage[:, 64 * c:64 * c + 64], in0=ps_os[h],
                        scalar=rcol[:, h:h + 1], in1=ubc_sb,
                        op0=Alu.mult, op1=Alu.add)
                    if c == 3:
                        nc.sync.dma_start(out_d[128 * g:128 * g + 128, 0:256],
                                          ostage[:, 0:256])
                nc.sync.dma_start(out_d[128 * g:128 * g + 128, 256:512],
                                  ostage[:, 256:512])
            return tail

        qaccs_by_lq = {}

        def mk_qaccs(lq):
            qaccs_by_lq[lq] = [
                small.tile([128, 4], f32, tag="qacc", name=f"qacc_{2*lq+j}")
                for j in range(2)]

        # prologue: lq0 projections emitted directly
        for ac in range(4):
            qk_chain(wk_sb, bk_sb, kT, 0, ac, on_act=True)
        mk_qaccs(0)
        for ac in range(4):
            qk_chain(wq_sb, bq_sb, qT, 0, ac, accs=qaccs_by_lq[0])
        kdup_fill(0)
        v_proj(0)
        v_proj(1)

        sms = {}

        def mk_preps(lq):
            sms[2 * lq] = p_sm.tile([128, 144], f32, tag="ps_sm",
                                    name=f"sm_{2*lq}")
            prep_group(2 * lq, sms[2 * lq], qaccs_by_lq[lq])
            sms[2 * lq + 1] = p_sm.tile([128, 144], f32, tag="ps_sm",
                                        name=f"sm_{2*lq+1}")
            prep_group(2 * lq + 1, sms[2 * lq + 1], qaccs_by_lq[lq])

        mk_preps(0)
        for lq in range(4):
            if lq < 2:
                w0 = 512 + 256 * lq
                nc.sync.dma_start(xT[:, :, w0:w0 + 256], x_src[:, :, w0:w0 + 256])
            if lq < 3:
                nlq = lq + 1
                mk_qaccs(nlq)
                kf = [(lambda ac=ac: qk_chain(wk_sb, bk_sb, kT, nlq, ac,
                                              on_act=True))
                      for ac in range(4)]
                qf = [(lambda ac=ac: qk_chain(wq_sb, bq_sb, qT, nlq, ac,
                                              accs=qaccs_by_lq[nlq]))
                      for ac in range(4)]
                fill0 = kf
                fill1 = (qf
                         + [lambda: mk_preps(nlq),
                            lambda: v_proj(2 * nlq), lambda: v_proj(2 * nlq + 1)])
            else:
                fill0, fill1 = [], []

            tail0 = main_group(2 * lq, sms[2 * lq], fill0)
            if lq < 3:
                kdup_fill(lq + 1)
            tail0()
            tail1 = main_group(2 * lq + 1, sms[2 * lq + 1], fill1)
            tail1()

    nc.compile()
    return nc


def _host_prep(inputs):
    import ml_dtypes
    bf = ml_dtypes.bfloat16
    f32 = np.float32
    g = {k: np.asarray(v, dtype=f32) for k, v in inputs.items()}
    Wq_eff = g["W_Q"] @ g["Wq_w"].T          # [E, A]
    Wk_eff = g["W_K"] @ g["Wk_w"].T
    Wv_eff = g["W_V"] @ g["Wv_w"].T
    Wu_eff = g["W_K"] @ g["Wu_w"].T          # [E, H]

    def chunk_w(w):  # [E, A] -> [128, 8, A] with [p, ec, a] = w[128*ec+p, a]
        return np.ascontiguousarray(
            w.reshape(8, 128, A).transpose(1, 0, 2)).astype(bf)

    wq, wk, wv = chunk_w(Wq_eff), chunk_w(Wk_eff), chunk_w(Wv_eff)
    bq = np.ascontiguousarray(g["Wq_b"].reshape(4, 128).T)
    bk = np.ascontiguousarray(g["Wk_b"].reshape(4, 128).T)
    bv = g["Wv_b"].reshape(1, A).astype(bf)

    x = g["x"]                                # [B, L, E]
    unary = np.einsum("ble,eh->blh", x, Wu_eff) + g["Wu_b"]
    unary -= unary.max(axis=1, keepdims=True)
    eu = np.exp(unary)
    uw = eu / eu.sum(axis=1, keepdims=True)   # [B, L, H]

    per_core = []
    for b in range(B):
        xT = np.ascontiguousarray(x[b].T).astype(bf)
        ucol = np.ascontiguousarray(
            uw[b].reshape(G, 128, H).transpose(1, 0, 2)).astype(bf)
        per_core.append(dict(xT=xT, Wq=wq, Wk=wk, Wv=wv, bq=bq, bk=bk,
                             bv=bv, ucol=ucol))
    return per_core


_NC_CACHE = {}


def kernel(**inputs):
    from concourse.bass_utils import run_bass_kernel_spmd

    if "nc" not in _NC_CACHE:
        _NC_CACHE["nc"] = _build_nc()
    nc = _NC_CACHE["nc"]

    in_maps = _host_prep(inputs)

    trace = os.environ.get("KERNEL_TRACE", "0") == "1"
    # First execution after a fresh NEFF load occasionally hits a transient
    # NRT_EXEC_UNIT_UNRECOVERABLE; a retry on the reloaded device succeeds.
    last_exc = None
    for _attempt in range(3):
        try:
            res = run_bass_kernel_spmd(nc, in_maps,
                                       core_ids=list(range(N_CORES)),
                                       trace=trace)
            break
        except Exception as e:
            last_exc = e
    else:
        raise last_exc
    if trace and res.exec_time_ns is not None:
        print(f"HW exec time: {res.exec_time_ns} ns")
        kernel.last_exec_time_ns = res.exec_time_ns
    out = np.stack([r["out"] for r in res.results], axis=0)
    return out
